# revision 7
# baseline (speedup 1.0000x reference)
"""DMoN head kernel for 8x Trainium2 NeuronCores (Bass/Tile, SPMD).

Strategy
--------
Only trace(S^T A S) is needed from the big adjacency contraction, and the
adjacency depends only on edge_index (an input), so the host builds the dense
0/1 adjacency (exact in bf16) and column-shards it over 8 cores.

Rotation trick: core c receives every row-indexed input rotated by c*BLK so
the single SPMD program uses only static slices -- each core's "first BLK
rows" are its own shard. Each core redundantly computes h = selu(emb@W^T+b),
logits, s = softmax (cheap), then contracts its A column-block:
  T = [s_hi|s_lo]^T @ A_blk   (bf16 split of s packed as one 128-wide
                               stationary -> fp32-exact at bf16 speed)
and reduces diag_k = sum_local s[local,k]*T[k,local] on-device.  The K-sized
side terms (ss = S^T S, S^T deg, cluster_size, M^T S) accumulate off the same
stationary into one PSUM bank.  Host sums the 8 per-core partials (the "KxK
all-reduce" of the sharding hint) and assembles the scalar losses.

TRI mode halves A traffic: keep each undirected pair once (wrap-distance rule,
core-independent under rotation), pack only live 512-wide blocks, and let the
host add the (tiny) self-loop term; trace = 2*sum(diag) + self_term.
"""

import os
import sys

import numpy as np

for _p in ("/opt/trn_rl_repo", "/root/.axon_site/_ro/trn_rl_repo"):
    if os.path.isdir(_p) and _p not in sys.path:
        sys.path.insert(0, _p)

import ml_dtypes  # noqa: E402

BF16 = ml_dtypes.bfloat16

LAM = 1.0507009873554805  # selu lambda
ALPHA = 1.6732632423543772  # selu alpha
NCORES = 8
NTYPES = 17
SMALLW = 147  # 128 (ss quads) + 2 (S^T[deg|1]) + 17 (S^T M)
TRI = os.environ.get("DMON_TRI", "1") == "1"

_nc_cache = {}


# ---------------------------------------------------------------------------
# static liveness for TRI mode: block of A_half rows r in [128g,128g+128),
# cols l in [512h, 512h+512) is live iff some (l-r) mod n lies in [1, n/2].
def _tri_live(ntot, g, h):
    lo = (512 * h - 128 * g - 127) % ntot
    width = 128 + 512 - 1
    for d in range(lo, lo + width + 1):
        dm = d % ntot
        if 1 <= dm <= ntot // 2:
            return True
    return False


def _live_blocks(ntot, blk):
    g_tiles = ntot // 128
    nh = max(1, blk // 512)
    out = []
    for g in range(g_tiles):
        for h in range(nh):
            if not TRI or _tri_live(ntot, g, h):
                out.append((g, h))
    return out


# ---------------------------------------------------------------------------
def _legalize_waits(nc, mybir):
    """This sandbox's walrus encodes at most ONE sync wait per instruction;
    Tile emits up to ~3. Hoist extra waits into standalone same-engine
    EventSemaphore (wait-only) instructions placed just before the owner."""
    n = 0
    for f in nc.m.functions:
        for blk in f.blocks:
            out = []
            for inst in blk.instructions:
                si = inst.sync_info
                if (si is not None and len(si.on_wait) > 1
                        and inst.opcode != "EventSemaphore"
                        and inst.engine != mybir.EngineType.Unassigned):
                    waits = list(si.on_wait)
                    for w in waits[:-1]:
                        n += 1
                        ev = mybir.InstEventSemaphore(
                            name=f"legw-{n}", engine=inst.engine,
                            sync_info=mybir.SyncInfo(on_wait=[w], on_update=[]))
                        nc.register_instruction(ev)
                        out.append(ev)
                    inst.sync_info = mybir.SyncInfo(
                        on_wait=[waits[-1]], on_update=list(si.on_update))
                out.append(inst)
            blk.instructions[:] = out
    return n


def build_bass(ntot, blk):
    """Build the SPMD Bass program (identical on all cores)."""
    import concourse.bass as bass
    import concourse.tile as tile
    from concourse import mybir

    f32 = mybir.dt.float32
    bf16 = mybir.dt.bfloat16
    AF = mybir.ActivationFunctionType
    OP = mybir.AluOpType

    G = ntot // 128          # contraction tiles
    NP = ntot // 512         # h panels
    NH = max(1, blk // 512)  # A column halves per core
    HW_ = min(blk, 512)      # A half width
    NOUT = blk // 128        # output row-tiles per core
    live = _live_blocks(ntot, blk)
    ncols_a = len(live) * HW_
    # DMA chunking for A (tile-major packed [128, ncols_a])
    ACH = 8192 if ncols_a % 8192 == 0 else HW_ * 2
    while ncols_a % ACH:
        ACH //= 2
    n_ach = ncols_a // ACH
    ECH = 1024 if ntot % 1024 == 0 else 512  # embT col chunk

    nc = bass.Bass()

    # inputs (per core, pre-rotated+packed by host)
    A_d = nc.declare_dram_parameter("a_pk", [128, ncols_a], bf16, isOutput=False)
    et_hi = nc.declare_dram_parameter("et_hi", [128, 2, ntot], bf16, isOutput=False)
    et_lo = nc.declare_dram_parameter("et_lo", [128, 2, ntot], bf16, isOutput=False)
    wt_hi = nc.declare_dram_parameter("wt_hi", [128, 2, 256], bf16, isOutput=False)
    wt_lo = nc.declare_dram_parameter("wt_lo", [128, 2, 256], bf16, isOutput=False)
    cen_d = nc.declare_dram_parameter("cen", [128, 2, 64], f32, isOutput=False)
    bex_d = nc.declare_dram_parameter("bexp", [128, 2], f32, isOutput=False)
    bre_d = nc.declare_dram_parameter("brelu", [128, 2], f32, isOutput=False)
    dg_d = nc.declare_dram_parameter("degones", [128, G, 2], bf16, isOutput=False)
    mt_d = nc.declare_dram_parameter("mtyp", [128, G, NTYPES], bf16, isOutput=False)
    id_d = nc.declare_dram_parameter("ident", [128, 128], f32, isOutput=False)

    # outputs
    lg_o = nc.declare_dram_parameter("logits_sh", [blk, 64], f32, isOutput=True)
    s_o = nc.declare_dram_parameter("s_sh", [blk, 64], f32, isOutput=True)
    dg_o = nc.declare_dram_parameter("diag", [64, NH], f32, isOutput=True)
    sm_o = nc.declare_dram_parameter("small", [128, SMALLW], f32, isOutput=True)

    with tile.TileContext(nc) as tc:
        with (
            tc.tile_pool(name="const", bufs=1) as cpool,
            tc.tile_pool(name="hT", bufs=1) as hpool,
            tc.tile_pool(name="achunk", bufs=3) as apool,
            tc.tile_pool(name="embc", bufs=3) as epool,
            tc.tile_pool(name="work", bufs=4) as wpool,
            tc.tile_pool(name="sfp", bufs=8) as spool,
            tc.tile_pool(name="hilo", bufs=8) as hlpool,
            tc.tile_pool(name="tail", bufs=8) as tpool,
            tc.tile_pool(name="hps", bufs=2, space=bass.MemorySpace.PSUM) as hps,
            tc.tile_pool(name="lps", bufs=3, space=bass.MemorySpace.PSUM) as lps,
            tc.tile_pool(name="accps", bufs=1, space=bass.MemorySpace.PSUM) as accps,
        ):
            # resident constants
            wthi = cpool.tile([128, 2, 256], bf16, tag="wthi")
            wtlo = cpool.tile([128, 2, 256], bf16, tag="wtlo")
            cen = cpool.tile([128, 2, 64], f32, tag="cen")
            bex = cpool.tile([128, 2], f32, tag="bex")
            bre = cpool.tile([128, 2], f32, tag="bre")
            dgo = cpool.tile([128, G, 2], bf16, tag="dgo")
            mty = cpool.tile([128, G, NTYPES], bf16, tag="mty")
            idn = cpool.tile([128, 128], f32, tag="idn")
            sth = cpool.tile([64, blk], f32, tag="sth")
            diag_sb = cpool.tile([64, NH], f32, tag="diag")
            for t_sb, t_d in ((wthi, wt_hi), (wtlo, wt_lo), (cen, cen_d),
                              (bex, bex_d), (bre, bre_d), (dgo, dg_d),
                              (mty, mt_d), (idn, id_d)):
                nc.sync.dma_start(t_sb[:], t_d[:])

            # persistent hT (f32, [hh-tile][128, ntot])
            hT = [hpool.tile([128, ntot], f32, tag=f"hT{t}", name=f"hT{t}")
                  for t in range(2)]

            # persistent PSUM accumulators
            TA = [accps.tile([128, HW_], f32, tag=f"TA{h}", name=f"TA{h}")
                  for h in range(NH)]
            smp = accps.tile([128, SMALLW], f32, tag="smp")

            # A-chunk schedule: live block i sits at packed cols [i*HW_,(i+1)*HW_)
            a_sb = None
            a_next = 0  # next live-block index not yet covered by DMA
            li = 0       # current live-block cursor
            last_g = {}
            for (g_, h_) in live:
                last_g[h_] = max(last_g.get(h_, 0), g_)

            et_hi_sb = et_lo_sb = None
            first_mm = {}  # (kind, h) -> bool for start flags

            for p in range(NP):
                # ---- embT chunk DMA (covers ECH cols) ----
                if (p * 512) % ECH == 0:
                    c0 = p * 512
                    et_hi_sb = epool.tile([128, 2, ECH], bf16, tag="ehi")
                    et_lo_sb = epool.tile([128, 2, ECH], bf16, tag="elo")
                    nc.sync.dma_start(et_hi_sb[:], et_hi[:, :, c0:c0 + ECH])
                    nc.sync.dma_start(et_lo_sb[:], et_lo[:, :, c0:c0 + ECH])
                co = (p * 512) % ECH  # col offset within chunk

                # ---- h panel: hT[t1][:, p*512:(p+1)*512] ----
                for t1 in range(2):
                    ph = hps.tile([128, 512], f32, tag="hps")
                    nmm = 0
                    for t0 in range(2):
                        for (w, e) in ((wthi, et_hi_sb), (wtlo, et_hi_sb),
                                       (wthi, et_lo_sb)):
                            nc.tensor.matmul(
                                ph[:],
                                w[:, t0, t1 * 128:(t1 + 1) * 128],
                                e[:, t0, co:co + 512],
                                start=(nmm == 0), stop=(nmm == 5),
                            )
                            nmm += 1
                    ex = wpool.tile([128, 512], f32, tag="ex")
                    nc.scalar.activation(ex[:], ph[:], AF.Exp,
                                         bias=bex[:, t1:t1 + 1], scale=1.0)
                    rl = wpool.tile([128, 512], f32, tag="rl")
                    nc.scalar.activation(rl[:], ph[:], AF.Relu,
                                         bias=bre[:, t1:t1 + 1], scale=1.0)
                    mn = wpool.tile([128, 512], f32, tag="mn")
                    nc.vector.tensor_scalar(mn[:], ex[:], ALPHA, -ALPHA,
                                            OP.min, OP.add)
                    nc.vector.tensor_add(hT[t1][:, p * 512:(p + 1) * 512],
                                         mn[:], rl[:])

                # ---- per 128-row tile: logits, softmax, hilo, A-chain ----
                for gi in range(4):
                    g = p * 4 + gi
                    lp = lps.tile([128, 128], f32, tag="lg")
                    nc.tensor.matmul(lp[:, 0:64],
                                     hT[0][:, g * 128:(g + 1) * 128],
                                     cen[:, 0, :], start=True, stop=False)
                    nc.tensor.matmul(lp[:, 0:64],
                                     hT[1][:, g * 128:(g + 1) * 128],
                                     cen[:, 1, :], start=False, stop=True)
                    lg = wpool.tile([128, 64], f32, tag="lgs")
                    nc.vector.tensor_scalar_mul(lg[:], lp[:, 0:64], LAM / 16.0)
                    ex2 = wpool.tile([128, 64], f32, tag="ex2")
                    rs = wpool.tile([128, 1], f32, tag="rs")
                    nc.scalar.activation(ex2[:], lg[:], AF.Exp, bias=0.0,
                                         scale=1.0, accum_out=rs[:])
                    rc = wpool.tile([128, 1], f32, tag="rc")
                    nc.vector.reciprocal(rc[:], rs[:])
                    sf = spool.tile([128, 64], f32, tag="sf")
                    nc.vector.tensor_scalar_mul(sf[:], ex2[:], rc[:])
                    hl = hlpool.tile([128, 128], bf16, tag="hl")
                    nc.vector.tensor_copy(hl[:, 0:64], sf[:])
                    nc.vector.tensor_sub(hl[:, 64:128], sf[:], hl[:, 0:64])

                    if g < NOUT:
                        nc.sync.dma_start(lg_o[g * 128:(g + 1) * 128, :], lg[:])
                        nc.sync.dma_start(s_o[g * 128:(g + 1) * 128, :], sf[:])
                        tp = lps.tile([64, 128], f32, tag="lg")
                        nc.tensor.transpose(tp[:], sf[:], idn[:])
                        nc.vector.tensor_copy(
                            sth[:, g * 128:(g + 1) * 128], tp[:])

                    # A-chain: T halves (only live blocks), then side terms
                    for h in range(NH):
                        if (g, h) not in set(live):
                            continue
                        if li >= a_next:  # need next A chunk
                            a_sb = apool.tile([128, ACH], bf16, tag="ach")
                            nc.sync.dma_start(
                                a_sb[:],
                                A_d[:, a_next * HW_:a_next * HW_ + ACH])
                            a_next += ACH // HW_
                        base = (li % (ACH // HW_)) * HW_
                        key = ("T", h)
                        nc.tensor.matmul(TA[h][:], hl[:],
                                         a_sb[:, base:base + HW_],
                                         start=not first_mm.get(key, False),
                                         stop=(g == last_g[h]),
                                         skip_group_check=True)
                        first_mm[key] = True
                        li += 1
                    nc.tensor.matmul(smp[:, 0:128], hl[:], hl[:],
                                     start=(g == 0), stop=False,
                                     skip_group_check=True)
                    nc.tensor.matmul(smp[:, 128:130], hl[:], dgo[:, g, :],
                                     start=False, stop=False,
                                     skip_group_check=True)
                    nc.tensor.matmul(smp[:, 130:SMALLW], hl[:], mty[:, g, :],
                                     start=False, stop=(g == G - 1),
                                     skip_group_check=True)

            # ---- tail: T = hi+lo, diag_k = sum_l T*s^T, copy small out ----
            for h in range(NH):
                lo_sb = tpool.tile([64, HW_], f32, tag="tl")
                nc.vector.tensor_copy(lo_sb[:], TA[h][64:128, :])
                t_sb = tpool.tile([64, HW_], f32, tag="tl")
                nc.vector.tensor_add(t_sb[:], TA[h][0:64, :], lo_sb[:])
                prod = tpool.tile([64, HW_], f32, tag="tl")
                nc.vector.tensor_mul(prod[:], t_sb[:],
                                     sth[:, h * HW_:(h + 1) * HW_])
                nc.vector.reduce_sum(diag_sb[:, h:h + 1], prod[:],
                                     axis=mybir.AxisListType.X)
            sm_sb = cpool.tile([128, SMALLW], f32, tag="smsb")
            nc.vector.tensor_copy(sm_sb[:], smp[:])
            nc.sync.dma_start(dg_o[:], diag_sb[:])
            nc.sync.dma_start(sm_o[:], sm_sb[:])

    _legalize_waits(nc, mybir)
    return nc


# ---------------------------------------------------------------------------
def _split_bf16(x):
    hi = x.astype(BF16)
    lo = (x - hi.astype(np.float32)).astype(BF16)
    return hi, lo


def host_prepare(embeddings, W_enc, b_enc, center_pool, edge_index,
                 joint_types, k):
    """Build adjacency-derived arrays + per-core rotated/packed in_maps."""
    ntot, d = embeddings.shape
    blk = ntot // NCORES
    G = ntot // 128
    HW_ = min(blk, 512)
    live = _live_blocks(ntot, blk)

    e0 = np.asarray(edge_index[0], np.int64) % ntot
    e1 = np.asarray(edge_index[1], np.int64) % ntot
    adj = np.zeros((ntot, ntot), dtype=np.uint8)
    adj[e0, e1] = 1
    adj |= adj.T  # symmetric 0/1, includes self-loops if present
    deg = adj.sum(axis=1, dtype=np.int64).astype(np.float32)
    dself = np.diagonal(adj).astype(np.float32)

    if TRI:
        # keep (i,j) once per unordered pair: d=(j-i) mod n in [1, n/2],
        # tie d==n/2 kept only for i<j; diagonal dropped (host adds it).
        jj = np.arange(ntot, dtype=np.int64)
        dmat = (jj[None, :] - jj[:, None]) % ntot
        keep = (dmat >= 1) & ((dmat < ntot // 2) |
                              ((dmat == ntot // 2) &
                               (jj[:, None] < jj[None, :])))
        a_use = (adj & keep).astype(BF16)
    else:
        a_use = adj.astype(BF16)

    embT = np.ascontiguousarray(embeddings.T.astype(np.float32))  # [256,ntot]
    wT = np.ascontiguousarray(W_enc.T.astype(np.float32))         # [256,256]
    wt_hi, wt_lo = _split_bf16(wT.reshape(2, 128, 256).transpose(1, 0, 2))
    cen = np.ascontiguousarray(
        center_pool[:64].T.astype(np.float32)).reshape(2, 128, 64)
    cen = np.ascontiguousarray(cen.transpose(1, 0, 2))
    b = np.asarray(b_enc, np.float32)
    bex = np.ascontiguousarray((b + np.log(ALPHA)).reshape(2, 128).T)
    bre = np.ascontiguousarray(b.reshape(2, 128).T)
    ident = np.eye(128, dtype=np.float32)
    onehot = (np.asarray(joint_types)[:, None] ==
              np.arange(NTYPES)[None, :]).astype(np.float32)

    in_maps = []
    for c in range(NCORES):
        rot = (np.arange(ntot) + c * blk) % ntot
        # A block: rows rotated, cols = original cols [c*blk, c*blk+blk)
        a_rot = np.concatenate([a_use[c * blk:], a_use[:c * blk]], axis=0)
        a_blk = a_rot[:, c * blk:c * blk + blk]          # [ntot, blk]
        a4 = a_blk.reshape(G, 128, blk // HW_, HW_)
        # pack live blocks tile-major: [128, nlive*HW_]
        a_pk = np.empty((128, len(live) * HW_), dtype=BF16)
        for i, (g, h) in enumerate(live):
            a_pk[:, i * HW_:(i + 1) * HW_] = a4[g, :, h, :]
        eT = embT[:, rot]
        ehi, elo = _split_bf16(eT.reshape(2, 128, ntot).transpose(1, 0, 2))
        dgo = np.stack([deg[rot].reshape(G, 128).T,
                        np.ones((128, G), np.float32)], axis=2).astype(BF16)
        mty = np.ascontiguousarray(
            onehot[rot].reshape(G, 128, NTYPES).transpose(1, 0, 2)).astype(BF16)
        in_maps.append({
            "a_pk": np.ascontiguousarray(a_pk),
            "et_hi": np.ascontiguousarray(ehi),
            "et_lo": np.ascontiguousarray(elo),
            "wt_hi": np.ascontiguousarray(wt_hi),
            "wt_lo": np.ascontiguousarray(wt_lo),
            "cen": cen, "bexp": bex, "brelu": bre,
            "degones": dgo, "mtyp": mty, "ident": ident,
        })
    return in_maps, deg, dself


def assemble(results, deg, dself, ntot, k):
    """Gather per-core outputs into the reference's return tuple."""
    blk = ntot // NCORES
    logits = np.concatenate([r["logits_sh"] for r in results], axis=0)
    s = np.concatenate([r["s_sh"] for r in results], axis=0)
    tr = np.float64(sum(float(r["diag"].sum()) for r in results))
    if TRI:
        selfrows = np.nonzero(dself > 0)[0]
        tr = 2.0 * tr + float((s[selfrows].astype(np.float64) ** 2).sum())
    sm = results[0]["small"].astype(np.float64)
    ss = sm[0:64, 0:64] + sm[0:64, 64:128] + sm[64:128, 0:64] + sm[64:128, 64:128]
    st_d = sm[0:64, 128] + sm[64:128, 128]
    csize = sm[0:64, 129] + sm[64:128, 129]
    typ = (sm[0:64, 130:SMALLW] + sm[64:128, 130:SMALLW]).T  # [17, 64]

    deg64 = deg.astype(np.float64)
    m = deg64.sum() / 2.0
    if m < 1e-8:
        spectral = 0.0
    else:
        null = (st_d @ st_d) / (2.0 * m)
        spectral = -(tr - null) / (2.0 * m)
    ssn = ss / (np.linalg.norm(ss) + 1e-8)
    i_k = np.eye(64) / np.sqrt(float(k))
    ortho = np.linalg.norm(ssn - i_k)
    cluster = np.sqrt(float(k)) / ntot * np.linalg.norm(csize) - 1.0
    type_loss = (np.maximum(typ - 1.0, 0.0) ** 2).sum()
    return (logits, s, np.float32(spectral), np.float32(ortho),
            np.float32(cluster), np.float32(type_loss))


# ---------------------------------------------------------------------------
def run(inputs, trace=False, tmpdir=None):
    from concourse.bass_utils import run_bass_kernel_spmd

    emb = np.asarray(inputs["embeddings"], np.float32)
    kk = int(np.asarray(inputs["k"]))
    assert kk == 64, f"kernel specialized for k=64, got {kk}"
    ntot = emb.shape[0]
    blk = ntot // NCORES

    in_maps, deg, dself = host_prepare(
        emb, inputs["W_enc"], inputs["b_enc"], inputs["center_pool"],
        inputs["edge_index"], inputs["joint_types"], kk)

    key = (ntot, blk, TRI)
    if key not in _nc_cache:
        _nc_cache[key] = build_bass(ntot, blk)
    nc = _nc_cache[key]

    br = run_bass_kernel_spmd(nc, in_maps, list(range(NCORES)),
                              trace=trace, tmpdir=tmpdir)
    outs = assemble(br.results, deg, dself, ntot, kk)
    return outs, br


def kernel(**inputs):
    outs, _ = run(inputs, trace=False)
    return outs
